# revision 11
# baseline (speedup 1.0000x reference)
"""AttnBlock (GroupNorm -> qkv 1x1 conv -> 8-head attention over 32x32
spatial -> proj 1x1 conv -> residual) on 8 Trainium2 NeuronCores.

Sharding: fully data-parallel, no collectives. Core i handles batch
b = i//2 and query-half s = i%2 (512 of the 1024 spatial positions).
Each core redundantly computes GroupNorm stats plus the full k/v
projections for its batch, then scores/softmax/AV/proj for its query
half. Host concatenates the per-core [512, 512] outputs.

v2 orchestration (vs the first working version):
  - Input DMA split over three rings (sync HWDGE: kvf/xs/xo; scalar
    HWDGE: weights, one 512KB transfer each; gpsimd SWDGE: constants)
    so the HBM load runs at fabric rate instead of one-queue rate.
  - PE warm-up matmuls during the DMA phase keep the HAM clock gate at
    8/8 so the first real matmuls run at 2.4 GHz, not 1.2.
  - The k/q/v projection tile emits for m>=1 are interleaved into
    attention tile 0's mk loop so the tensor engine fills the gaps it
    would otherwise spend waiting on the scalar-engine Exp.
  - PSUM evacuations split between ACT (early, with free bias via
    activation) and DVE (steady state, tensor_scalar_add) to keep the
    scalar engine dedicated to Exp during attention.
  - softmax denominators: ones-column in v^T makes Z appear in psum row
    64 of each AV accumulation; 1/Z = exp(-ln Z); tiles 0-2 batch their
    Ln/Exp after tile 2, tile 3 runs its own short chain so the tail is
    just its own z-dependency.
  - proj runs k-major: m=0,1 accumulate in ps_mm as on-tiles appear;
    m=2,3 run at the end in the freed score psum banks.

Toolchain workarounds: the Tile-tail Drain and any instruction carrying
more than one semaphore wait are rejected by this walrus build, so
excess waits are spread onto same-engine NoOps post-schedule.
"""

import os

import numpy as np

import concourse.bass as bass
import concourse.tile as tile
from concourse import mybir
from concourse.bass_utils import run_bass_kernel_spmd
from concourse.vector_clock import ScopedClock

# ---------------------------------------------------------------------------
# walrus workaround: the Tile kernel-tail Drain may carry more sem waits than
# the CTRL instruction encoding allows; spread them over sync-engine NOPs.
_MAX_WAITS_PER_INST = 1


def _patched_drain_and_barrier(self, tick_clock, wait_clock):
    nc = self.nc
    probe = nc.sync.nop(nofuse=True, hint="drain_wait_spread")
    wait_clock.add_sem_waits(probe.ins, ScopedClock({None: tick_clock.global_clock}))
    si = probe.ins.sync_info
    waits = list(si.on_wait) if si is not None else []
    if len(waits) > _MAX_WAITS_PER_INST:
        probe.ins.sync_info = mybir.SyncInfo(
            on_wait=waits[:_MAX_WAITS_PER_INST], on_update=[]
        )
        for i in range(_MAX_WAITS_PER_INST, len(waits), _MAX_WAITS_PER_INST):
            nop = nc.sync.nop(nofuse=True, hint="drain_wait_spread")
            nop.ins.sync_info = mybir.SyncInfo(
                on_wait=waits[i : i + _MAX_WAITS_PER_INST], on_update=[]
            )
    nc.sync.drain()
    nc.all_engine_barrier(sem_only=True)
    popped = nc._tile_sem_poison_stack.pop()
    assert popped is self._sem_poison
    nc.clear_and_free_semaphores(list(self.sems.allocated().values()))


tile.TileContext._drain_and_barrier = _patched_drain_and_barrier


def _split_multi_waits(nc, max_waits=1):
    """walrus rejects instructions with more than one sem wait; move the
    excess onto same-engine NoOps placed immediately before."""
    ctr = 0
    for blk in nc.m.functions[0].blocks:
        out = []
        for inst in blk.instructions:
            si = inst.sync_info
            waits = list(si.on_wait) if (si and si.on_wait) else []
            if len(waits) > max_waits:
                extra, keep = waits[:-max_waits], waits[-max_waits:]
                for j in range(0, len(extra), max_waits):
                    ctr += 1
                    nop = mybir.InstNoOp(name=f"I-wsplit-{ctr}")
                    nop.engine = inst.engine
                    nop.sync_info = mybir.SyncInfo(
                        on_wait=extra[j : j + max_waits], on_update=[])
                    out.append(nop)
                inst.sync_info = mybir.SyncInfo(
                    on_wait=keep,
                    on_update=list(si.on_update) if si.on_update else [])
            out.append(inst)
        blk.instructions = out
    return ctr
# ---------------------------------------------------------------------------

B = 4
C = 512
H = W = 32
HWF = 1024  # keys / full spatial
Q = 512  # queries per core (half of HWF)
NH = 8
CHD = 64  # channels per head
CT = 4  # 128-channel tiles of C
KT = 8  # 128-key tiles of HWF
GROUPS = 32
GPC = 16  # channels per group
EPS = 1e-6
F32 = mybir.dt.float32
BF16 = mybir.dt.bfloat16

_DT_NAME = os.environ.get("BASS_ATTN_DT", "bf16")
DT = {"f32": mybir.dt.float32, "bf16": mybir.dt.bfloat16,
      "f32r": mybir.dt.float32r}[_DT_NAME]

N_WARM = int(os.environ.get("BASS_ATTN_WARM", "20"))


def build_program():
    nc = bass.Bass("TRN2", target_bir_lowering=False, debug=False, num_devices=8)

    def din(name, shape, dt=F32):
        return nc.declare_dram_parameter(name, list(shape), dt, isOutput=False)

    xs_d = din("xs", [C, Q])
    xo_d = din("xo", [C, Q], BF16)
    kvf_d = din("kvf", [C, HWF], BF16)
    wq_d = din("wqT", [C, C], DT)
    wk_d = din("wkT", [C, C], DT)
    wv_d = din("wvT", [C, C], DT)
    wp_d = din("wpT", [C, C], DT)
    bv_d = din("bv", [C])
    cpack_d = din("cpack", [128, 36])
    e16_d = din("e16", [8, 128])
    eh8_d = din("eh8", [8, 512], DT)
    eh2_d = din("eh2", [2, 128], DT)
    out_d = nc.declare_dram_parameter("out", [C, Q], F32, isOutput=True)

    from contextlib import ExitStack
    with tile.TileContext(nc) as tc, ExitStack() as ctx:
        cst = ctx.enter_context(tc.tile_pool(name="cst", bufs=1))
        big = ctx.enter_context(tc.tile_pool(name="big", bufs=1))
        wrk = ctx.enter_context(tc.tile_pool(name="wrk", bufs=3))
        epool = ctx.enter_context(tc.tile_pool(name="epool", bufs=4))
        ps_s = ctx.enter_context(tc.tile_pool(name="ps_s", bufs=2, space="PSUM"))
        ps_o = ctx.enter_context(tc.tile_pool(name="ps_o", bufs=1, space="PSUM"))
        ps_mm = ctx.enter_context(tc.tile_pool(name="ps_mm", bufs=2, space="PSUM"))

        # ---- constants / small inputs (SWDGE ring, does not block HWDGE) ----
        cpk = cst.tile([128, 36], F32)
        nc.gpsimd.dma_start(cpk[:], cpack_d[:])
        bq_c, bk_c, bp_c = cpk[:, 0:4], cpk[:, 4:8], cpk[:, 8:12]
        gqs_c, gqb_c = cpk[:, 12:16], cpk[:, 16:20]
        gks_c, gkb_c = cpk[:, 20:24], cpk[:, 24:28]
        g16 = cpk[:, 28:36]
        e16 = cst.tile([8, 128], F32)
        nc.gpsimd.dma_start(e16[:], e16_d[:])
        eh8 = cst.tile([8, 512], DT)
        nc.gpsimd.dma_start(eh8[:], eh8_d[:])
        eh2 = cst.tile([2, 128], DT)
        nc.gpsimd.dma_start(eh2[:], eh2_d[:])
        bv_ap = bv_d[:]
        bvbc = cst.tile([128, C], F32)
        nc.gpsimd.dma_start(
            out=bvbc[:],
            in_=bass.AP(tensor=bv_ap.tensor, offset=bv_ap.offset,
                        ap=[[0, 128]] + list(bv_ap.ap)),
        )

        # ---- weights + xo on the ACT HWDGE ring (kv path first) ----
        w_sb = {}

        def wdma(wd, key):
            t_ = big.tile([128, 4 * C], DT, name=f"w_{key}")
            src = wd[:].rearrange("(k p) m -> k p m", p=128)
            for k in range(CT):
                nc.scalar.dma_start(t_[:, k * C : (k + 1) * C], src[k])
            w_sb[key] = t_

        def wchunk(key, k):  # [128, C] chunk of channel rows 128k..128k+127
            return w_sb[key][:, k * C : (k + 1) * C]

        wdma(wk_d, "k")
        wdma(wv_d, "v")
        xo = []
        for t in range(CT):
            xot = big.tile([128, Q], BF16, name=f"xo{t}")
            nc.scalar.dma_start(
                xot[:], xo_d[:].rearrange("(m p) q -> m p q", p=128)[t])
            xo.append(xot)
        wdma(wq_d, "q")
        wdma(wp_d, "p")

        # ---- big inputs on the sync HWDGE ring: kvf first (gates GN) ----
        kvf = []
        for t in range(CT):
            kt_ = big.tile([128, HWF], BF16, name=f"kvf{t}")
            nc.sync.dma_start(
                kt_[:], kvf_d[:].rearrange("(m p) q -> m p q", p=128)[t])
            kvf.append(kt_)
        xs = []
        for t in range(CT):
            xst = big.tile([128, Q], F32, name=f"xs{t}")
            nc.sync.dma_start(xst[:], xs_d[:].rearrange("(m p) q -> m p q", p=128)[t])
            xs.append(xst)

        # ---- PE warm-up: junk matmuls during the DMA phase keep HAM at 8/8
        junk = cst.tile([128, 256], BF16)
        nc.vector.memset(junk[:], 0.125)
        for i in range(N_WARM):
            pw = ps_mm.tile([128, 512], F32, name=f"pw{i}", tag="mm")
            nc.tensor.matmul(pw[:, 0:256], lhsT=junk[:, 0:128], rhs=junk[:],
                             start=True, stop=True)

        # ---- groupnorm affine coefficients (a, b per channel) ----
        def gn_coeffs(statc, gam, bet, label):
            gps = ps_mm.tile([128, 512], F32, name=f"gps_{label}", tag="mm")
            nc.tensor.matmul(gps[0:8, 0:8], lhsT=g16, rhs=statc[:],
                             start=True, stop=True)
            gs = wrk.tile([8, 8], F32, name=f"gs_{label}", tag="gs")
            nc.vector.tensor_copy(gs[:], gps[0:8, 0:8])
            ms = wrk.tile([8, 8], F32, name=f"ms_{label}", tag="ms")
            nc.vector.tensor_scalar_mul(ms[:], gs[:], 1.0 / GPC)
            msq8 = wrk.tile([8, 4], F32, name=f"msq8_{label}", tag="msq8")
            nc.vector.tensor_mul(msq8[:], ms[:, 0:4], ms[:, 0:4])
            var8 = wrk.tile([8, 4], F32, name=f"var8_{label}", tag="var8")
            nc.vector.tensor_sub(var8[:], ms[:, 4:8], msq8[:])
            # rstd = exp(-0.5*ln(var+eps)) — keeps ACT on one table set
            lnv = wrk.tile([8, 4], F32, name=f"lnv_{label}", tag="lnv")
            eps8 = wrk.tile([8, 1], F32, name=f"eps8_{label}", tag="eps8")
            nc.vector.memset(eps8[:], EPS)
            nc.scalar.activation(lnv[:], var8[:],
                                 mybir.ActivationFunctionType.Ln, bias=eps8[:])
            rhs2 = wrk.tile([8, 8], F32, name=f"rhs2_{label}", tag="rhs2", bufs=1)
            nc.scalar.activation(rhs2[:, 0:4], lnv[:],
                                 mybir.ActivationFunctionType.Exp, scale=-0.5)
            nc.vector.tensor_copy(rhs2[:, 4:8], ms[:, 0:4])
            pcs = ps_mm.tile([128, 512], F32, name=f"pcs_{label}", tag="mm")
            nc.tensor.matmul(pcs[:, 0:8], lhsT=e16[:], rhs=rhs2[:],
                             start=True, stop=True)
            pc = wrk.tile([128, 8], F32, name=f"pc_{label}", tag="pc")
            nc.vector.tensor_copy(pc[:], pcs[:, 0:8])
            a = wrk.tile([128, 4], F32, name=f"a_{label}", bufs=1)
            nc.vector.tensor_mul(a[:], pc[:, 0:4], gam)
            tmpb = wrk.tile([128, 4], F32, name=f"tmpb_{label}", tag="tmpb")
            nc.vector.tensor_mul(tmpb[:], pc[:, 4:8], a[:])
            b = wrk.tile([128, 4], F32, name=f"b_{label}", bufs=1)
            nc.vector.tensor_sub(b[:], bet, tmpb[:])
            return a, b

        # kv groupnorm: one bn_stats per [128, 1024] tile
        statk = wrk.tile([128, 8], F32, name="statk", bufs=1)
        for t in range(CT):
            bnst = wrk.tile([128, 2, 6], F32, name="bnst_kv", tag="bnst")
            nc.vector.bn_stats(out=bnst[:, 0, :], in_=kvf[t][:, 0:512])
            nc.vector.bn_stats(out=bnst[:, 1, :], in_=kvf[t][:, 512:1024])
            mv = wrk.tile([128, 2], F32, name="mv_kv", tag="mv")
            nc.vector.bn_aggr(out=mv[:], in_=bnst[:])
            nc.vector.tensor_copy(statk[:, t : t + 1], mv[:, 0:1])
            msq = wrk.tile([128, 1], F32, name="msq_kv", tag="msq")
            nc.vector.tensor_mul(msq[:], mv[:, 0:1], mv[:, 0:1])
            nc.vector.tensor_add(statk[:, 4 + t : 5 + t], msq[:], mv[:, 1:2])
        akv, bkv = gn_coeffs(statk, gks_c, gkb_c, "kv")

        kvn = []
        for t in range(CT):
            kn = big.tile([128, HWF], DT, name=f"kvn{t}")
            nc.vector.tensor_scalar(
                out=kn[:], in0=kvf[t][:],
                scalar1=akv[:, t : t + 1], scalar2=bkv[:, t : t + 1],
                op0=mybir.AluOpType.mult, op1=mybir.AluOpType.add)
            kvn.append(kn)

        # x groupnorm (stats over both halves: xs f32 + xo bf16)
        statx = wrk.tile([128, 8], F32, name="statx", bufs=1)
        for t in range(CT):
            bnst = wrk.tile([128, 2, 6], F32, name="bnst_x", tag="bnst")
            nc.vector.bn_stats(out=bnst[:, 0, :], in_=xs[t][:])
            nc.vector.bn_stats(out=bnst[:, 1, :], in_=xo[t][:])
            mv = wrk.tile([128, 2], F32, name="mv_x", tag="mv")
            nc.vector.bn_aggr(out=mv[:], in_=bnst[:])
            nc.vector.tensor_copy(statx[:, t : t + 1], mv[:, 0:1])
            msq = wrk.tile([128, 1], F32, name="msq_x", tag="msq")
            nc.vector.tensor_mul(msq[:], mv[:, 0:1], mv[:, 0:1])
            nc.vector.tensor_add(statx[:, 4 + t : 5 + t], msq[:], mv[:, 1:2])
        ax, bx = gn_coeffs(statx, gqs_c, gqb_c, "x")

        qin = []
        for t in range(CT):
            qt = big.tile([128, Q], DT, name=f"qin{t}")
            nc.vector.tensor_scalar(
                out=qt[:], in0=xs[t][:],
                scalar1=ax[:, t : t + 1], scalar2=bx[:, t : t + 1],
                op0=mybir.AluOpType.mult, op1=mybir.AluOpType.add)
            qin.append(qt)

        k_sb = [None] * CT
        q_sb = [None] * CT
        vT_sb = [None] * KT

        _pre_ps = [("oA", ps_o), ("oB", ps_o), ("mm", ps_mm), ("mm", ps_mm)]
        _pre_i = [0]

        def qkv_ps(name, during):
            if during:
                return ps_mm.tile([128, 512], F32, name=name, tag="mm")
            tag, pool = _pre_ps[_pre_i[0] % 4]
            _pre_i[0] += 1
            return pool.tile([128, 512], F32, name=name, tag=tag)

        def emit_k(m, during):
            kt_ = big.tile([128, HWF], DT, name=f"k{m}")
            for nh in range(2):
                ps = qkv_ps(f"psk{m}{nh}", during)
                for k in range(CT):
                    nc.tensor.matmul(
                        ps[:], lhsT=wchunk("k", k)[:, bass.ts(m, 128)],
                        rhs=kvn[k][:, bass.ts(nh, 512)],
                        start=(k == 0), stop=(k == CT - 1))
                if during:
                    nc.vector.tensor_scalar_add(
                        kt_[:, bass.ts(nh, 512)], ps[:], bk_c[:, m : m + 1])
                else:
                    nc.scalar.activation(kt_[:, bass.ts(nh, 512)], ps[:],
                                         mybir.ActivationFunctionType.Identity,
                                         bias=bk_c[:, m : m + 1])
            k_sb[m] = kt_

        def emit_q(m, during):
            ps = qkv_ps(f"psq{m}", during)
            for k in range(CT):
                nc.tensor.matmul(ps[:], lhsT=wchunk("q", k)[:, bass.ts(m, 128)],
                                 rhs=qin[k][:], start=(k == 0),
                                 stop=(k == CT - 1))
            qt = big.tile([128, Q], DT, name=f"q{m}")
            if during:
                nc.vector.tensor_scalar_add(qt[:], ps[:], bq_c[:, m : m + 1])
            else:
                nc.scalar.activation(qt[:], ps[:],
                                     mybir.ActivationFunctionType.Identity,
                                     bias=bq_c[:, m : m + 1])
            q_sb[m] = qt

        def emit_v(mt, during):
            vt = big.tile([128, NH * (CHD + 1)], DT, name=f"vT{mt}")
            ones_col = vt[:].rearrange("p (h c) -> p h c", c=CHD + 1)[
                :, :, CHD : CHD + 1]
            if DT == mybir.dt.float32r:
                ones_col = ones_col.bitcast(F32)
            nc.vector.memset(ones_col, 1.0)
            ps = qkv_ps(f"psv{mt}", during)
            for k in range(CT):
                nc.tensor.matmul(
                    ps[:], lhsT=kvn[k][:, bass.ts(mt, 128)],
                    rhs=wchunk("v", k), start=(k == 0), stop=(k == CT - 1))
            nc.vector.tensor_tensor(
                out=vt[:].rearrange("p (h c) -> p h c", c=CHD + 1)[:, :, 0:CHD],
                in0=ps[:].rearrange("p (h c) -> p h c", c=CHD),
                in1=bvbc[:].rearrange("p (h c) -> p h c", c=CHD),
                op=mybir.AluOpType.add)
            vT_sb[mt] = vt

        emit_k(0, False)
        emit_v(0, False)
        emit_v(1, False)
        emit_q(0, False)

        # emits drip-fed into attention's mk loop (one per slot)
        pending = (
            [("v", 2), ("v", 3), ("v", 4), ("v", 5), ("v", 6), ("v", 7),
             ("k", 1), ("q", 1), ("k", 2), ("q", 2), ("k", 3), ("q", 3)])

        def pop_emit():
            if pending:
                kind, i = pending.pop(0)
                if kind == "v":
                    emit_v(i, True)
                elif kind == "k":
                    emit_k(i, True)
                else:
                    emit_q(i, True)

        # ---- attention (head pairs t: heads 2t rows 0:64, 2t+1 rows 64:128)
        rz = wrk.tile([6, 512], F32, name="rz", bufs=1)
        rzL = wrk.tile([2, 512], F32, name="rzL", bufs=1)
        rzbE = wrk.tile([8, 512], DT, name="rzbE", bufs=1)
        nc.vector.memset(rzbE[:], 0.0)
        osts = [None] * CT
        on_sb = [None] * CT
        proj_ps = [None] * CT
        # post-z work queue: filled at t==2 (tiles 0-2) and t==3 (tile 3)
        post = []

        def zmul_proj(t, rzb, lhsT_z):
            zps = ps_s.tile([128, 1024], F32, name=f"zps{t}", tag="s")
            nc.tensor.matmul(zps[:, 0:512], lhsT=lhsT_z, rhs=rzb,
                             start=True, stop=True)
            ont = big.tile([128, Q], DT, name=f"on{t}")
            nc.vector.tensor_mul(ont[:], osts[t][:], zps[:, 0:512])
            on_sb[t] = ont
            for m in range(2):
                if proj_ps[m] is None:
                    proj_ps[m] = ps_mm.tile([128, 512], F32, name=f"psp{m}",
                                            tag="mm")

        def proj_mm(t):
            for m in range(2):
                nc.tensor.matmul(
                    proj_ps[m][:], lhsT=wchunk("p", t)[:, bass.ts(m, 128)],
                    rhs=on_sb[t][:], start=(t == 0), stop=(t == CT - 1))

        for t in range(CT):
            poA = ps_o.tile([128, 512], F32, name=f"poA{t}", tag="oA")
            poB = ps_o.tile([128, 512], F32, name=f"poB{t}", tag="oB")
            for mk in range(KT):
                pss = ps_s.tile([128, 1024], F32, name=f"pss{t}{mk}", tag="s")
                nc.tensor.matmul(pss[:, 0:512],
                                 lhsT=k_sb[t][0:64, bass.ts(mk, 128)],
                                 rhs=q_sb[t][0:64, :],
                                 start=True, stop=True, tile_position=(0, 0))
                nc.tensor.matmul(pss[:, 512:1024],
                                 lhsT=k_sb[t][64:128, bass.ts(mk, 128)],
                                 rhs=q_sb[t][64:128, :],
                                 start=True, stop=True, tile_position=(64, 0))
                et = epool.tile([128, 1024], DT, name=f"e{t}{mk}", tag="e")
                nc.scalar.activation(et[:], pss[:],
                                     mybir.ActivationFunctionType.Exp,
                                     scale=float(CHD) ** -0.5)
                nc.tensor.matmul(poA[0:65, :],
                                 lhsT=vT_sb[mk][:, bass.ds(130 * t, 65)],
                                 rhs=et[:, 0:512],
                                 start=(mk == 0), stop=(mk == KT - 1))
                nc.tensor.matmul(poB[0:65, :],
                                 lhsT=vT_sb[mk][:, bass.ds(130 * t + 65, 65)],
                                 rhs=et[:, 512:1024],
                                 start=(mk == 0), stop=(mk == KT - 1))
                pop_emit()
                if post:
                    post.pop(0)()
            # evacuate AV psums; head B shifts to rows 64:128 via DMA
            ost = wrk.tile([128, 512], F32, name=f"ost{t}", tag="ost", bufs=4)
            nc.vector.tensor_copy(ost[0:64, :], poA[0:64, :])
            zst = wrk.tile([65, 512], F32, name=f"zst{t}", tag="zst", bufs=4)
            nc.vector.tensor_copy(zst[64:65, :], poA[64:65, :])
            stB = wrk.tile([128, 512], F32, name=f"stB{t}", tag="stB", bufs=4)
            nc.vector.tensor_copy(stB[0:65, :], poB[0:65, :])
            if t < 3:
                nc.sync.dma_start(rz[2 * t : 2 * t + 1, :], zst[64:65, :])
                nc.sync.dma_start(rz[2 * t + 1 : 2 * t + 2, :], stB[64:65, :])
            else:
                nc.sync.dma_start(rzL[0:1, :], zst[64:65, :])
                nc.sync.dma_start(rzL[1:2, :], stB[64:65, :])
            nc.sync.dma_start(ost[64:128, :], stB[0:64, :])
            osts[t] = ost
            if t == 2:
                def _early():
                    lnE = wrk.tile([6, 512], F32, name="lnE", bufs=1)
                    nc.scalar.activation(lnE[:], rz[0:6, :],
                                         mybir.ActivationFunctionType.Ln)
                    nc.scalar.activation(rzbE[0:6, :], lnE[:],
                                         mybir.ActivationFunctionType.Exp,
                                         scale=-1.0)
                post = [
                    _early,
                    lambda: zmul_proj(0, rzbE[:], eh8[:, 0:128]),
                    lambda: proj_mm(0),
                    lambda: zmul_proj(1, rzbE[:], eh8[:, 128:256]),
                    lambda: proj_mm(1),
                    lambda: zmul_proj(2, rzbE[:], eh8[:, 256:384]),
                    lambda: proj_mm(2),
                ]
        for fn in post:
            fn()
        # tile 3: short z chain right after its AV completes
        lnL = wrk.tile([2, 512], F32, name="lnL", bufs=1)
        nc.scalar.activation(lnL[:], rzL[:], mybir.ActivationFunctionType.Ln)
        rzbL = wrk.tile([2, 512], DT, name="rzbL", bufs=1)
        nc.scalar.activation(rzbL[:], lnL[:], mybir.ActivationFunctionType.Exp,
                             scale=-1.0)
        zmul_proj(3, rzbL[:], eh2[:])
        proj_mm(3)

        # proj m=2,3 in the freed score psum banks, then residual + store
        for m in range(2, CT):
            ps = ps_s.tile([128, 1024], F32, name=f"psp{m}", tag="s")
            proj_ps[m] = ps
            for k in range(CT):
                nc.tensor.matmul(
                    ps[:, 0:512], lhsT=wchunk("p", k)[:, bass.ts(m, 128)],
                    rhs=on_sb[k][:], start=(k == 0), stop=(k == CT - 1))

        for m in range(CT):
            src = proj_ps[m][:] if m < 2 else proj_ps[m][:, 0:512]
            r1 = wrk.tile([128, Q], F32, name=f"r1_{m}", tag="r1")
            nc.scalar.activation(r1[:], src,
                                 mybir.ActivationFunctionType.Identity,
                                 bias=bp_c[:, m : m + 1])
            r2 = wrk.tile([128, Q], F32, name=f"r2_{m}", tag="r2")
            nc.vector.tensor_add(r2[:], r1[:], xs[m][:])
            nc.sync.dma_start(
                out_d[:].rearrange("(m p) q -> m p q", p=128)[m], r2[:])

    _split_multi_waits(nc)
    return nc


_NC_CACHE = None
LAST_EXEC_NS = None


def _np_dt():
    if DT == mybir.dt.bfloat16:
        import ml_dtypes
        return ml_dtypes.bfloat16
    return np.float32


def kernel(**inputs):
    global _NC_CACHE, LAST_EXEC_NS
    x = np.asarray(inputs["x"], dtype=np.float32)
    kv = np.asarray(inputs["kv"], dtype=np.float32)
    wdt = _np_dt()
    wqT = np.ascontiguousarray(np.asarray(inputs["wq"], np.float32).T).astype(wdt)
    wkT = np.ascontiguousarray(np.asarray(inputs["wk"], np.float32).T).astype(wdt)
    wvT = np.ascontiguousarray(np.asarray(inputs["wv"], np.float32).T).astype(wdt)
    wpT = np.ascontiguousarray(np.asarray(inputs["wproj"], np.float32).T).astype(wdt)
    bq = np.asarray(inputs["bq"], np.float32)
    bk = np.asarray(inputs["bk"], np.float32)
    bv = np.asarray(inputs["bv"], np.float32)
    bp = np.asarray(inputs["bproj"], np.float32)
    gqs = np.asarray(inputs["gnq_scale"], np.float32)
    gqb = np.asarray(inputs["gnq_bias"], np.float32)
    gks = np.asarray(inputs["gnkv_scale"], np.float32)
    gkb = np.asarray(inputs["gnkv_bias"], np.float32)

    p = np.arange(128)
    g16 = (p[:, None] // GPC == np.arange(8)[None, :]).astype(np.float32)
    e16 = np.ascontiguousarray(g16.T)
    eh8 = (np.arange(512)[None, :] // CHD == np.arange(8)[:, None]).astype(
        _np_dt())
    eh2 = (np.arange(128)[None, :] // CHD == np.arange(2)[:, None]).astype(
        _np_dt())
    cpack = np.concatenate(
        [v.reshape(4, 128).T for v in (bq, bk, bp, gqs, gqb, gks, gkb)]
        + [g16], axis=1).astype(np.float32)
    cpack = np.ascontiguousarray(cpack)

    xr = x.reshape(B, C, HWF)
    kvr = kv.reshape(B, C, HWF)

    in_maps = []
    for core in range(8):
        b, s = core // 2, core % 2
        import ml_dtypes
        in_maps.append({
            "xs": np.ascontiguousarray(xr[b][:, s * Q : (s + 1) * Q]),
            "xo": np.ascontiguousarray(
                xr[b][:, (1 - s) * Q : (2 - s) * Q]).astype(ml_dtypes.bfloat16),
            "kvf": np.ascontiguousarray(kvr[b]).astype(ml_dtypes.bfloat16),
            "wqT": wqT, "wkT": wkT, "wvT": wvT, "wpT": wpT,
            "bv": bv, "cpack": cpack, "e16": e16, "eh8": eh8, "eh2": eh2,
        })

    if _NC_CACHE is None:
        _NC_CACHE = build_program()

    trace = os.environ.get("BASS_ATTN_TRACE", "0") == "1"
    res = run_bass_kernel_spmd(_NC_CACHE, in_maps, core_ids=list(range(8)),
                               trace=trace)
    LAST_EXEC_NS = res.exec_time_ns
    globals()["LAST_RES"] = res

    out = np.empty((B, C, HWF), np.float32)
    for core in range(8):
        b, s = core // 2, core % 2
        out[b][:, s * Q : (s + 1) * Q] = res.results[core]["out"]
    return out.reshape(B, C, H, W)
